# revision 1
# baseline (speedup 1.0000x reference)
"""Trainium2 Bass kernel for nn_IntrinsicReward (retrieval_knn).

Computes, for B=1024 samples:
  pred_err  = mean((MLP(concat(z_t, action)) - z_t1)^2, -1)   (tiny 3-layer MLP w/ LayerNorm)
  epistemic = mean(sigma, -1)
  novelty   = clip(1 - mean(top16(cos_sim(z_t, mem))), 0, 1)  (kNN over M=200000 memories)
  total     = pred_err + 0.5*epistemic + 0.5*novelty
returns stack([total, pred_err, epistemic, novelty])  -> (4, B) f32

Distribution (8 NeuronCores):
  - mem is sharded along M (25000 rows/core). Host pre-normalizes mem rows
    (exact f32), transposes to (D, M_shard), zero-pads M to a multiple of 512
    and casts bf16 (layout prep for the TensorE contraction layout).
  - Each core computes its local (B x M_shard) similarity matrix on TensorE
    (bf16), reduces it on-chip to per-16-chunk maxima (split between VectorE
    windowed reduce directly from PSUM and ScalarE copy + VectorE max-tree),
    then extracts the local top-8 per row with the DVE max8 unit.
  - The tiny MLP is data-parallel over batch: core c handles rows
    [128c, 128c+128).
  - Host gathers the 8x(B,8) local candidates and re-reduces the global
    top-16 (standard distributed kNN merge), then combines the reward terms.
"""

import numpy as np

import concourse.bass as bass
import concourse.mybir as mybir
from concourse import bacc, tile
from concourse.bass_utils import run_bass_kernel_spmd
from concourse.masks import make_identity

# ---------------------------------------------------------------- constants
B, D, A, M, K = 1024, 256, 6, 200000, 16
H1, H2 = 128, 64
LN_EPS = 1e-5
W_PRED, W_EPIST, W_NOVEL = 1.0, 0.5, 0.5

NCORES = 8
MLOC = M // NCORES            # 25000 memories per core
SC = 1024                      # "superchunk": matmul cols per PSUM (2 banks)
NSC = MLOC // SC               # 24 full superchunks
TAIL = MLOC - NSC * SC         # 424 -> padded to 512
TAILP = 512
MPAD = NSC * SC + TAILP        # 25088
SLAB = 6                       # superchunks per DMA slab
NSLAB = NSC // SLAB            # 4
CH = 16                        # chunk size for chunk-max candidates
NCMX = MPAD // CH              # 1568 chunk maxima per row per core
A_SC = (1, 2, 3, 4, 5)         # superchunks within a slab on the ACT path
V_SC = (0,)                    # superchunks on the direct-DVE path

F32 = mybir.dt.float32
BF16 = mybir.dt.bfloat16
NPBF16 = mybir.dt.np(BF16)

_CACHE = {}
DEBUG_CMX = False
BUILD_LEVEL = 3  # 0: prelude only, 1: +1 slab, 2: +all slabs/tail, 3: full


# ---------------------------------------------------------------- program
def build_program():
    """Build + compile the SPMD Bass program (identical on all 8 cores)."""
    nc = bacc.Bacc(
        "TRN2", target_bir_lowering=False, debug=False, num_devices=NCORES
    )

    din = {}
    dout = {}

    def inp(name, shape, dt):
        din[name] = nc.dram_tensor(name, list(shape), dt, kind="ExternalInput").ap()
        return din[name]

    # inputs (per core)
    d_memT = inp("memT", (2, 128, MPAD), BF16)       # sharded, normalized, T
    d_z = inp("z", (B, D), F32)                      # replicated raw z_t
    d_zb = inp("zb", (128, D), F32)                  # this core's batch slice
    d_zt1 = inp("zt1", (128, D), F32)
    d_sigma = inp("sigma", (128, A), F32)
    d_actT = inp("actT", (A, 128), F32)
    d_w1k0 = inp("w1k0", (128, H1), F32)
    d_w1k1 = inp("w1k1", (128, H1), F32)
    d_w1a = inp("w1a", (A, H1), F32)
    d_w2 = inp("w2", (H1, H2), F32)
    d_w3 = inp("w3", (H2, D), F32)
    d_b1r = inp("b1r", (128, H1), F32)
    d_g1r = inp("g1r", (128, H1), F32)
    d_be1r = inp("be1r", (128, H1), F32)
    d_b2r = inp("b2r", (128, H2), F32)
    d_g2r = inp("g2r", (128, H2), F32)
    d_be2r = inp("be2r", (128, H2), F32)
    d_b3r = inp("b3r", (128, D), F32)

    # outputs
    d_loc8 = nc.dram_tensor("loc8", [NCORES, 128, 8], F32, kind="ExternalOutput").ap()
    d_pe2 = nc.dram_tensor("pe2", [128, 2], F32, kind="ExternalOutput").ap()
    dout["loc8"] = d_loc8
    dout["pe2"] = d_pe2

    X = mybir.AxisListType.X
    OP = mybir.AluOpType
    AF = mybir.ActivationFunctionType

    with tile.TileContext(nc) as tc:
        with (
            tc.tile_pool(name="const", bufs=1) as cpool,
            tc.tile_pool(name="sbuf", bufs=2) as spool,
            tc.tile_pool(name="psum", bufs=3, space="PSUM") as mmpool,
            tc.tile_pool(name="psx", bufs=2, space="PSUM") as auxpool,
        ):
            # ---------------- constants / weights ----------------
            ident = cpool.tile([128, 128], F32, tag="ident")
            make_identity(nc, ident[:])

            czero = cpool.tile([128, 1], F32, tag="czero")
            nc.vector.memset(czero[:], 0.0)
            nc.const_aps.aps[(F32, 0.0)] = czero[:]

            def load_const(dram, shape, dt, tag):
                t = cpool.tile(list(shape), dt, tag=tag)
                nc.sync.dma_start(out=t[:], in_=dram)
                return t

            w1k0 = load_const(d_w1k0, (128, H1), F32, "w1k0")
            w1k1 = load_const(d_w1k1, (128, H1), F32, "w1k1")
            w1a = load_const(d_w1a, (A, H1), F32, "w1a")
            w2 = load_const(d_w2, (H1, H2), F32, "w2")
            w3 = load_const(d_w3, (H2, D), F32, "w3")
            b1r = load_const(d_b1r, (128, H1), F32, "b1r")
            g1r = load_const(d_g1r, (128, H1), F32, "g1r")
            be1r = load_const(d_be1r, (128, H1), F32, "be1r")
            b2r = load_const(d_b2r, (128, H2), F32, "b2r")
            g2r = load_const(d_g2r, (128, H2), F32, "g2r")
            be2r = load_const(d_be2r, (128, H2), F32, "be2r")
            b3r = load_const(d_b3r, (128, D), F32, "b3r")
            actT = load_const(d_actT, (A, 128), F32, "actT")
            zb = load_const(d_zb, (128, D), F32, "zb")
            zt1 = load_const(d_zt1, (128, D), F32, "zt1")
            sigma = load_const(d_sigma, (128, A), F32, "sigma")

            # full z, all 8 batch tiles: partition p holds rows {p, p+128, ...}
            z_all = cpool.tile([128, 8, D], F32, tag="z_all")
            nc.sync.dma_start(
                out=z_all[:], in_=d_z.rearrange("(a p) d -> p a d", p=128)
            )

            # zT_norm: z/(||z||+1e-8), transposed, bf16. two K-tiles (D=2*128)
            zTn = [
                cpool.tile([128, B], BF16, tag=f"zTn{k}", name=f"zTn{k}")
                for k in range(2)
            ]

            small = cpool.tile([128, 8], F32, tag="small")  # norm scratch
            for bt in range(8):
                ss = small[:, bt : bt + 1]
                sq = spool.tile([128, D], F32, tag="zsq")
                nc.scalar.activation(
                    out=sq[:], in_=z_all[:, bt], func=AF.Square, accum_out=ss
                )
                nc.scalar.activation(out=ss, in_=ss, func=AF.Sqrt)
                nc.vector.tensor_scalar_add(ss, ss, 1e-8)
                nc.vector.reciprocal(ss, ss)
                zn = spool.tile([128, D], F32, tag="zn")
                nc.vector.tensor_scalar_mul(zn[:], z_all[:, bt], ss)
                for k in range(2):
                    ps = auxpool.tile([128, 128], F32, tag="aux")
                    nc.tensor.transpose(ps[:], zn[:, 128 * k : 128 * (k + 1)], ident[:])
                    nc.vector.tensor_copy(zTn[k][:, 128 * bt : 128 * (bt + 1)], ps[:])

            # ---------------- tiny MLP on this core's batch slice ----------
            # lhsT for layer 1: transpose of raw zb (2 blocks of 128)
            zbT = cpool.tile([128, 2, 128], F32, tag="zbT")
            for k in range(2):
                ps = auxpool.tile([128, 128], F32, tag="aux")
                nc.tensor.transpose(ps[:], zb[:, 128 * k : 128 * (k + 1)], ident[:])
                nc.vector.tensor_copy(zbT[:, k], ps[:])

            def layernorm_relu(h_psum, bias_r, g_r, be_r, width, out_bf_T):
                """x = h_psum + bias_r; y = relu(LN(x)*g+be); return yT (bf16)
                via PE transpose. h_psum: (128, width) PSUM f32."""
                x = spool.tile([128, width], F32, tag=f"ln_x{width}")
                nc.vector.tensor_tensor(
                    out=x[:], in0=h_psum[:], in1=bias_r[:], op=OP.add
                )
                st = spool.tile([128, 6], F32, tag=f"ln_st{width}")
                nc.vector.bn_stats(st[:], x[:])
                st2 = spool.tile([128, 2], F32, tag=f"ln_st2{width}")
                nc.vector.bn_aggr(st2[:], st[:])
                sd = spool.tile([128, 1], F32, tag=f"ln_sd{width}")
                nc.vector.tensor_scalar_add(sd[:], st2[:, 1:2], LN_EPS)
                nc.scalar.activation(out=sd[:], in_=sd[:], func=AF.Sqrt)
                nc.vector.reciprocal(sd[:], sd[:])
                xh = spool.tile([128, width], F32, tag=f"ln_xh{width}")
                nc.vector.tensor_scalar(
                    out=xh[:],
                    in0=x[:],
                    scalar1=st2[:, 0:1],
                    scalar2=sd[:],
                    op0=OP.subtract,
                    op1=OP.mult,
                )
                nc.vector.tensor_tensor(out=xh[:], in0=xh[:], in1=g_r[:], op=OP.mult)
                nc.vector.tensor_tensor(out=xh[:], in0=xh[:], in1=be_r[:], op=OP.add)
                nc.vector.tensor_scalar_max(xh[:], xh[:], 0.0)
                pst = auxpool.tile([128, 128], F32, tag="aux")
                nc.tensor.transpose(pst[:width, :], xh[:], ident[:])
                nc.vector.tensor_copy(out_bf_T[:], pst[:width, :128])

            h1 = auxpool.tile([128, H1], F32, tag="aux")
            nc.tensor.matmul(h1[:], zbT[:, 0], w1k0[:], start=True, stop=False)
            nc.tensor.matmul(h1[:], zbT[:, 1], w1k1[:], start=False, stop=False)
            nc.tensor.matmul(h1[:], actT[:], w1a[:], start=False, stop=True)
            h1T = cpool.tile([H1, 128], F32, tag="h1T")
            layernorm_relu(h1, b1r, g1r, be1r, H1, h1T)

            h2 = auxpool.tile([128, H2], F32, tag="aux")
            nc.tensor.matmul(h2[:], h1T[:], w2[:], start=True, stop=True)
            h2T = cpool.tile([H2, 128], F32, tag="h2T")
            layernorm_relu(h2, b2r, g2r, be2r, H2, h2T)

            zp = auxpool.tile([128, D], F32, tag="aux")
            nc.tensor.matmul(zp[:], h2T[:], w3[:], start=True, stop=True)

            pe2 = cpool.tile([128, 2], F32, tag="pe2")
            diff = spool.tile([128, D], F32, tag="diff")
            nc.vector.tensor_tensor(out=diff[:], in0=zp[:], in1=b3r[:], op=OP.add)
            nc.vector.tensor_tensor(out=diff[:], in0=diff[:], in1=zt1[:], op=OP.subtract)
            dsq = spool.tile([128, D], F32, tag="dsq")
            # Square((x/16)) accumulated over D -> sum(x^2)/256 = mean(x^2)
            nc.scalar.activation(
                out=dsq[:], in_=diff[:], func=AF.Square, scale=1.0 / 16.0,
                accum_out=pe2[:, 0:1],
            )
            nc.vector.reduce_sum(out=pe2[:, 1:2], in_=sigma[:], axis=X)
            nc.vector.tensor_scalar_mul(pe2[:, 1:2], pe2[:, 1:2], 1.0 / A)
            nc.sync.dma_start(out=d_pe2, in_=pe2[:])

            # ---------------- main kNN loop -------------------------------
            # chunk maxima for all 8 batch tiles: (128, 8, NCMX) bf16
            cmx = cpool.tile([128, 8, NCMX], BF16, tag="cmx")
            if BUILD_LEVEL < 2:
                nc.vector.memset(cmx[:], -2.0)

            nslab_run = 0 if BUILD_LEVEL < 1 else (1 if BUILD_LEVEL == 1 else NSLAB)
            for s in range(nslab_run):
                slab = spool.tile([128, 2, SLAB * SC], BF16, tag="slab")
                for k in range(2):
                    nc.sync.dma_start(
                        out=slab[:, k],
                        in_=d_memT[k, :, s * SLAB * SC : (s + 1) * SLAB * SC],
                    )
                for bt in range(8):
                    stA = spool.tile([128, len(A_SC) * SC], BF16, tag="stA")
                    n_a = 0
                    for j in range(SLAB):
                        ps = mmpool.tile([128, SC], F32, tag="mm")
                        for half in range(2):
                            col = j * SC + half * 512
                            for k in range(2):
                                nc.tensor.matmul(
                                    ps[:, half * 512 : (half + 1) * 512],
                                    zTn[k][:, 128 * bt : 128 * (bt + 1)],
                                    slab[:, k, col : col + 512],
                                    start=(k == 0),
                                    stop=(k == 1),
                                )
                        cm = cmx[
                            :, bt, (s * SLAB + j) * (SC // CH):
                            (s * SLAB + j + 1) * (SC // CH)
                        ]
                        if j in V_SC:
                            nc.vector.tensor_reduce(
                                out=cm,
                                in_=ps[:].rearrange("p (w c) -> p w c", c=CH),
                                axis=X,
                                op=OP.max,
                            )
                        else:
                            nc.scalar.copy(
                                out=stA[:, n_a * SC : (n_a + 1) * SC], in_=ps[:]
                            )
                            n_a += 1
                    # max-tree over the staged ACT-path superchunks
                    na = len(A_SC) * SC          # 5120
                    v0 = stA[:].rearrange("p (w c) -> p w c", c=16)
                    t1 = spool.tile([128, na // 2], BF16, tag="t1")
                    v1 = t1[:].rearrange("p (w c) -> p w c", c=8)
                    nc.vector.tensor_tensor(
                        out=v1, in0=v0[:, :, :8], in1=v0[:, :, 8:], op=OP.max
                    )
                    t2 = spool.tile([128, na // 4], BF16, tag="t2")
                    v2 = t2[:].rearrange("p (w c) -> p w c", c=4)
                    nc.vector.tensor_tensor(
                        out=v2, in0=v1[:, :, :4], in1=v1[:, :, 4:], op=OP.max
                    )
                    t3 = spool.tile([128, na // 8], BF16, tag="t3")
                    v3 = t3[:].rearrange("p (w c) -> p w c", c=2)
                    nc.vector.tensor_tensor(
                        out=v3, in0=v2[:, :, :2], in1=v2[:, :, 2:], op=OP.max
                    )
                    cmA = cmx[
                        :, bt,
                        (s * SLAB) * (SC // CH) + len(V_SC) * (SC // CH):
                        (s * SLAB + SLAB) * (SC // CH)
                    ]
                    nc.vector.tensor_tensor(
                        out=cmA, in0=v3[:, :, 0], in1=v3[:, :, 1], op=OP.max
                    )

            # tail superchunk (512 padded cols), direct V path
            tailt = spool.tile([128, 2, TAILP], BF16, tag="tail")
            if BUILD_LEVEL >= 2:
                for k in range(2):
                    nc.sync.dma_start(
                        out=tailt[:, k], in_=d_memT[k, :, NSC * SC : NSC * SC + TAILP]
                    )
            for bt in range(8 if BUILD_LEVEL >= 2 else 0):
                ps = auxpool.tile([128, TAILP], F32, tag="aux", name="tailp")
                for k in range(2):
                    nc.tensor.matmul(
                        ps[:],
                        zTn[k][:, 128 * bt : 128 * (bt + 1)],
                        tailt[:, k],
                        start=(k == 0),
                        stop=(k == 1),
                    )
                cm = cmx[:, bt, NSC * (SC // CH) : NCMX]
                nc.vector.tensor_reduce(
                    out=cm,
                    in_=ps[:].rearrange("p (w c) -> p w c", c=CH),
                    axis=X,
                    op=OP.max,
                )

            # ---------------- local top-8 per batch tile ------------------
            loc8b = cpool.tile([128, 8, 8], BF16, tag="loc8b")
            for bt in range(8):
                nc.vector.max(out=loc8b[:, bt], in_=cmx[:, bt])
            loc8f = cpool.tile([128, 8, 8], F32, tag="loc8f")
            nc.vector.tensor_copy(loc8f[:], loc8b[:])
            nc.sync.dma_start(
                out=d_loc8.rearrange("a p k -> p a k"), in_=loc8f[:]
            )

            if DEBUG_CMX:
                d_cmx = nc.dram_tensor(
                    "dbg_cmx", [128, 8, NCMX], BF16, kind="ExternalOutput"
                ).ap()
                nc.sync.dma_start(out=d_cmx, in_=cmx[:])

    nc.compile()
    return nc


def _prep(inputs):
    """Host-side sharding/layout prep. Returns per-core input maps."""
    f32 = np.float32
    z = np.asarray(inputs["z_t"], f32)
    action = np.asarray(inputs["action"], f32)
    z_t1 = np.asarray(inputs["z_t1"], f32)
    sigma = np.asarray(inputs["sigma"], f32)
    mem = np.asarray(inputs["mem"], f32)
    W1 = np.asarray(inputs["W1"], f32)
    W2 = np.asarray(inputs["W2"], f32)
    W3 = np.asarray(inputs["W3"], f32)
    b1 = np.asarray(inputs["b1"], f32)
    g1 = np.asarray(inputs["g1"], f32)
    be1 = np.asarray(inputs["be1"], f32)
    b2 = np.asarray(inputs["b2"], f32)
    g2 = np.asarray(inputs["g2"], f32)
    be2 = np.asarray(inputs["be2"], f32)
    b3 = np.asarray(inputs["b3"], f32)

    # normalize memory rows exactly in f32 (part of sharding/layout prep)
    mem_n = mem / (np.linalg.norm(mem, axis=-1, keepdims=True) + 1e-8)

    rep = lambda v, w: np.ascontiguousarray(np.broadcast_to(v[None, :], (128, w)), f32)

    common = {
        "z": np.ascontiguousarray(z),
        "w1k0": np.ascontiguousarray(W1[:128]),
        "w1k1": np.ascontiguousarray(W1[128:256]),
        "w1a": np.ascontiguousarray(W1[256:262]),
        "w2": W2,
        "w3": W3,
        "b1r": rep(b1, H1),
        "g1r": rep(g1, H1),
        "be1r": rep(be1, H1),
        "b2r": rep(b2, H2),
        "g2r": rep(g2, H2),
        "be2r": rep(be2, H2),
        "b3r": rep(b3, D),
    }

    in_maps = []
    for c in range(NCORES):
        sl = slice(c * 128, (c + 1) * 128)
        shard = mem_n[c * MLOC : (c + 1) * MLOC]          # (25000, 256)
        memT = np.zeros((2, 128, MPAD), NPBF16)
        sT = np.ascontiguousarray(shard.T.astype(NPBF16))  # (256, 25000)
        memT[0, :, :MLOC] = sT[:128]
        memT[1, :, :MLOC] = sT[128:]
        in_maps.append(
            dict(
                common,
                memT=memT,
                zb=np.ascontiguousarray(z[sl]),
                zt1=np.ascontiguousarray(z_t1[sl]),
                sigma=np.ascontiguousarray(sigma[sl]),
                actT=np.ascontiguousarray(action[sl].T),
            )
        )
    return in_maps


def _merge(results):
    """Host-side gather + global top-16 re-reduce + reward combine."""
    cand = np.concatenate(
        [np.asarray(r["loc8"], np.float32).reshape(B, 8) for r in results], axis=1
    )  # (B, 64)
    top16 = np.sort(cand, axis=1)[:, -K:]
    novelty = np.clip(1.0 - top16.mean(axis=1), 0.0, 1.0).astype(np.float32)
    pred = np.concatenate([r["pe2"][:, 0] for r in results])
    epist = np.concatenate([r["pe2"][:, 1] for r in results])
    total = W_PRED * pred + W_EPIST * epist + W_NOVEL * novelty
    return np.stack([total, pred, epist, novelty], axis=0).astype(np.float32)


def run_on_hw(in_maps, trace=False):
    if "nc" not in _CACHE:
        _CACHE["nc"] = build_program()
    res = run_bass_kernel_spmd(
        _CACHE["nc"], in_maps, list(range(NCORES)), trace=trace
    )
    return res


def kernel(**inputs) -> np.ndarray:
    in_maps = _prep(inputs)
    res = run_on_hw(in_maps)
    return _merge(res.results)

